# revision 44
# baseline (speedup 1.0000x reference)
"""Trainium2 Bass kernel for nn_MPCActor: MLP (256->512->512->8-useful-cols,
relu/relu/sigmoid) followed by the closed-form equivalent of 100 SGD steps on
u (u_N = A*u0 + (1-A)*c, A = (1-2*lr*q)^100, c = -p/(2q); x_init never
enters the u-gradient).

Data parallel over 8 NeuronCores: batch 32768 -> 4096 rows per core, weights
replicated, batch tiles processed in pairs. All L1/L2 matmuls run in
fp8(e4m3) with perf_mode=DoubleRow (K=256 contracted per instruction, 2x
FLOPs at the PE's 1 col/cycle streaming rate). The obs transpose, weight
layouts, fp8 casts and scale folding are all done on the host:
  obsT = obs.T/8 (fp8)      W1h = 8*W1 (fp8)   -> psum1 = z1 exactly
  y1   = relu(z1+b1) (fp8)  W2h = 64*W2 (fp8)  -> psum2 = 64*z2
  y2'  = relu(psum2+64*b2) = 64*y2 (fp8)
  W3h  = 16*W3[:, useful 8 cols]               -> psum3 = 1024*z3
  qp   = sigmoid(psum3/1024 + b3)              (one fused ACT op per QUAD)

Startup is DMA-latency-tuned (each HWDGE ring delivers its first ~128KB
about 2us after the billing clock starts, then ~1.9us per DMA): obs tiles
ride the SP ring in consumption order with the small loads slotted into
slack, w1+obs1 ride the ACT ring ahead of the activation-table loads, w2
halves ride GPSIMD SWDGE as a third ring, and a burst of junk matmuls
(uninitialized-value reads into an overwritten psum bank) keeps the PE
HAM-busy through the first-DMA latency so real matmuls run at 2.4GHz.
Pair 0 runs fully tile-major (t0's L1+L2 while obs1 is in flight, t0's L1
drains DVE-only while the ACT tables load).  PSUM is 8 banks: 6 rotating
single-bank y-groups (one drain frees a bank ~0.56us after its matmul,
alternating ACT relu / DVE max) + the quad z3 bank + the transpose bank.
All FOUR tiles of two consecutive pairs (a quad) share one L3 accumulation
group (four 64-wide column-shifted W3 variants -> one [64,512] bank), so a
single sigmoid + four [64,128] PE transposes serve four batch tiles.
Closed form: c = -p/(2q) and
  A = (1-.02q)^100 = exp(-(2q+.02q^2))*(1+O(3e-4)) = sigmoid(-w)/sigmoid(w)
  where w+50 = (0.1414214*q + 7.0710678)^2  (one ACT Square, biases from
  SBUF consts; both sigmoids hit the already-loaded sigmoid table set).
Reciprocals use the ~5x-faster 18-bit reciprocal_approx_fast custom-DVE op.
The overlapped (non-final) quad runs its chain on the otherwise-idle GPSIMD
(A by repeated squaring, 100 = 64+32+4); the final quad is latency-split
across ACT (square+sigmoids straight from psum) and DVE (fast recips +
finish), and the output stores are bf16 (~1e-3 extra rel err,
well under the 2e-2 gate) split across both HWDGE rings.  The walrus-emitted
epilogue (~8.6us: queue drains, barriers, and a serial clear of all 256
semaphores) and the ~2us first-DMA latency are fixed costs this kernel
cannot remove.
"""

import numpy as np
import ml_dtypes

import concourse.mybir as mybir
import concourse.tile as tile
from concourse import bacc
from concourse.bass_utils import run_bass_kernel_spmd

NCORES = 8
BATCH = 32768
BPC = BATCH // NCORES  # 4096 rows per core
OBS = 256
HID = 512
NQP = 16  # q_u (4) + p_u (4) + 8 zero-pad cols (step%16==0 for DoubleRow)
BT = 512  # batch tile (matmul moving free dim)
NT = BPC // BT  # 8 batch tiles per core
LR = 0.01
NWARM = 14  # junk matmuls (N=256) covering first-DMA latency + HAM warm-up
F32 = mybir.dt.float32
FP8 = mybir.dt.float8e4
BF16 = mybir.dt.bfloat16
FP16 = mybir.dt.float16
DR = mybir.MatmulPerfMode.DoubleRow

_CACHE = {}


def _build_nc(zero_bias):
    nc = bacc.Bacc(
        trn_type="TRN2", target_bir_lowering=False, debug=False, num_devices=NCORES
    )
    obsT = nc.declare_dram_parameter("obsT", [NT, 128, 2, BT], FP8, isOutput=False).ap()
    u0 = nc.declare_dram_parameter("u0", [NT // 4, 128, 4, 4, 4], F32, isOutput=False).ap()
    w1 = nc.declare_dram_parameter("w1", [128, 2, HID], FP8, isOutput=False).ap()
    w2 = nc.declare_dram_parameter("w2", [128, 4, HID], FP8, isOutput=False).ap()
    w3 = nc.declare_dram_parameter("w3", [128, 4, 256], FP8, isOutput=False).ap()
    b1 = nc.declare_dram_parameter("b1", [128, 4], F32, isOutput=False).ap()
    b2 = nc.declare_dram_parameter("b2", [128, 4], F32, isOutput=False).ap()
    b3 = nc.declare_dram_parameter("b3", [64, 1], F32, isOutput=False).ap()
    idm = nc.declare_dram_parameter("idm", [64, 64], F32, isOutput=False).ap()
    uo = nc.declare_dram_parameter("uo", [NT // 4, 128, 4, 4, 4], BF16, isOutput=True).ap()

    AF = mybir.ActivationFunctionType
    ALU = mybir.AluOpType

    with tile.TileContext(nc) as tc:
        from contextlib import ExitStack

        with ExitStack() as ctx:
            singles = ctx.enter_context(tc.tile_pool(name="singles", bufs=1))
            p_y1 = ctx.enter_context(tc.tile_pool(name="y1", bufs=2))
            p_y2 = ctx.enter_context(tc.tile_pool(name="y2", bufs=2))
            p_qp = ctx.enter_context(tc.tile_pool(name="qp", bufs=2))
            p_cf = ctx.enter_context(tc.tile_pool(name="cf", bufs=2))
            # PSUM budget is 8 banks: y 6x1 + z3 quad-group 1 + psq 1
            pp_y = ctx.enter_context(tc.tile_pool(name="ppy", bufs=6, space="PSUM"))
            pp_q = ctx.enter_context(tc.tile_pool(name="ppq", bufs=1, space="PSUM"))

            # ---- junk tile for PE warm-up (memset first on the otherwise
            # idle GPSIMD so the Tensor queue barely waits) ----
            junk = singles.tile([128, 2, 256], FP8)
            nc.gpsimd.memset(junk, 1.0)
            # per-partition constants for the tail's ACT bias operands
            cb_sq = singles.tile([128, 1], F32, name="cb_sq")
            nc.gpsimd.memset(cb_sq, 7.0710678)
            cb_p50 = singles.tile([128, 1], F32, name="cb_p50")
            nc.gpsimd.memset(cb_p50, 50.0)
            cb_m50 = singles.tile([128, 1], F32, name="cb_m50")
            nc.gpsimd.memset(cb_m50, -50.0)

            # ---- input DMAs, latency-ordered.  SP ring: the eight obs
            # tiles in consumption order with the small loads (b3/idm/u0)
            # slotted into the slack; ACT ring: w1, then the two w2 halves,
            # then (behind the act-table loads) the late obs are NOT here --
            # all obs stay on SP so the table loads never delay them. ----
            obst = [singles.tile([128, 2, BT], FP8, name=f"obs{t}") for t in range(NT)]
            w1s = singles.tile([128, 2, HID], FP8)
            w2s = singles.tile([128, 4, HID], FP8)
            w3s = singles.tile([128, 4, 256], FP8)
            b3s = singles.tile([64, 1], F32)
            ids = singles.tile([64, 64], F32)

            nc.sync.dma_start(out=obst[0], in_=obsT[0])
            nc.scalar.dma_start(out=w1s, in_=w1)
            nc.scalar.dma_start(out=obst[1], in_=obsT[1])
            # w2 rides GPSIMD SWDGE (third ring; GPSIMD is idle at startup)
            nc.gpsimd.dma_start(out=w2s[:, 0:2, :], in_=w2[:, 0:2, :])
            nc.gpsimd.dma_start(out=w2s[:, 2:4, :], in_=w2[:, 2:4, :])
            nc.sync.dma_start(out=obst[2], in_=obsT[2])
            nc.sync.dma_start(out=obst[3], in_=obsT[3])
            nc.sync.dma_start(out=obst[4], in_=obsT[4])
            nc.sync.dma_start(out=b3s, in_=b3, single_packet=True)
            nc.sync.dma_start(out=ids, in_=idm, single_packet=True)
            u0b = {}
            for G in range(NT // 4):
                u0b[G] = p_cf.tile([128, 4, 4, 4], F32, name=f"u0b{G}", tag=f"u0b{G}")
            nc.sync.dma_start(out=obst[5], in_=obsT[5])
            nc.sync.dma_start(out=u0b[0], in_=u0[0], single_packet=True)
            nc.sync.dma_start(out=obst[6], in_=obsT[6])
            nc.sync.dma_start(out=obst[7], in_=obsT[7])
            nc.sync.dma_start(out=u0b[1], in_=u0[1], single_packet=True)
            if zero_bias:
                b1s = b2s = None
            else:
                b1s = singles.tile([128, 4], F32)
                nc.sync.dma_start(out=b1s, in_=b1)
                b2s = singles.tile([128, 4], F32)
                nc.sync.dma_start(out=b2s, in_=b2)

            # dummy sigmoid on ACT (after the ACT ring's DMA issues) so the
            # sigmoid-capable activation table set (which also contains
            # Relu) loads at startup, off the drain critical path
            dum = singles.tile([128, 16], F32)
            nc.scalar.activation(
                out=dum, in_=junk[:, 0, 0:16], func=AF.Sigmoid, bias=0.0, scale=1.0
            )
            nc.scalar.dma_start(out=w3s, in_=w3)

            # ---- PE warm-up: junk matmuls, one shared stationary ----
            wps = pp_y.tile([128, BT], F32, name="wps", tag="y")
            for i in range(NWARM):
                nc.tensor.matmul(
                    wps[:, 0:256],
                    junk[:, 0:2, 0:128],
                    junk,
                    start=True,
                    stop=True,
                    perf_mode=DR,
                )

            def drain1(dst, src, bias_sb, m, on_act):
                # dst [128, 512] fp8 <- relu(src [128, 512] single-bank psum
                # + bias); one drain per bank keeps the 6-deep rotation
                # smooth and frees each bank ~0.56us after its last matmul.
                b = None if zero_bias else bias_sb[:, m : m + 1]
                if on_act:
                    nc.scalar.activation(
                        out=dst,
                        in_=src,
                        func=AF.Relu,
                        bias=0.0 if zero_bias else b,
                        scale=1.0,
                    )
                elif zero_bias:
                    nc.vector.tensor_scalar(dst, src, 0.0, None, ALU.max)
                else:
                    nc.vector.tensor_scalar(dst, src, b, 0.0, ALU.add, ALU.max)

            def tail(G, qpT, u0g):
                # transpose to batch-major + closed form + store, for quad G
                # (4 batch tiles). Emitted AFTER the next pair's layer-1
                # matmuls so the PE never idles on the sigmoid.
                last = G == NT // 4 - 1
                if last:
                    psq4 = pp_q.tile([128, 4, 64], F32, tag="psq")
                    for c in range(4):
                        nc.tensor.transpose(
                            psq4[:, c, :], qpT[:, c * 128 : (c + 1) * 128], ids[:]
                        )
                else:
                    # overlapped quad: transpose via the DMA XBAR (16-bit
                    # only, hence the bf16 qpT) -- saves ~1.1us of PE time;
                    # emitted after all obs issues so it never delays them
                    psq4 = p_cf.tile([128, 4, 64], BF16, tag="psqS")
                    for c in range(4):
                        nc.sync.dma_start(
                            out=psq4[:, c, :],
                            in_=qpT[:, c * 128 : (c + 1) * 128],
                            transpose=True,
                        )
                psq = psq4.rearrange("x c (u j) -> x c u j", u=4)
                q = psq[:, :, :, 0:4]
                p = psq[:, :, :, 4:8]
                SH = [128, 4, 4, 4]
                SH8 = [128, 4, 4, 8]
                if last:
                    # latency-split chain.  ACT (reads psum directly):
                    #   yq = (0.1414q+7.071)^2 = .02q^2+2q+50 = w+50
                    #   s2 = sigmoid(yq-50) = sigmoid(w); s1 = sigmoid(-w)
                    # DVE: fast reciprocals (18-bit, plenty) + finish;
                    # GPSIMD: the c-branch products.
                    yq = p_cf.tile(SH, F32, tag="yq")
                    nc.scalar.activation(
                        out=yq, in_=q, func=AF.Square, bias=cb_sq[:, 0:1],
                        scale=0.14142136,
                    )
                    s2 = p_cf.tile(SH, F32, tag="s2")  # sigmoid(w)
                    nc.scalar.activation(
                        out=s2, in_=yq, func=AF.Sigmoid, bias=cb_m50[:, 0:1],
                        scale=1.0,
                    )
                    s1 = p_cf.tile(SH, F32, tag="s1")  # sigmoid(-w)
                    nc.scalar.activation(
                        out=s1, in_=yq, func=AF.Sigmoid, bias=cb_p50[:, 0:1],
                        scale=-1.0,
                    )
                    p8 = psq[:, :, :, 4:8]
                    rq8 = p_cf.tile(SH, F32, tag="rq8")  # 1/q
                    nc.vector.reciprocal_approx_fast(
                        rq8.rearrange("x c u j -> x (c u) j"),
                        psq4.rearrange("x c (u j) -> x (c u) j", u=4)[:, :, 0:4],
                    )
                    s0 = rq8
                    rr = p_cf.tile(SH, F32, tag="rr")  # 1/sigmoid(w)
                    nc.vector.reciprocal_approx_fast(
                        rr.rearrange("x c u j -> x (c u) j"),
                        s2.rearrange("x c u j -> x (c u) j"),
                    )
                    A = p_cf.tile(SH, F32, tag="A")  # e^-w = s1/s2
                    nc.vector.tensor_mul(A, s1, rr)
                    cc = p_cf.tile(SH, F32, tag="cc")  # -p/(2q)
                    nc.vector.scalar_tensor_tensor(cc, p8, -0.5, s0, ALU.mult, ALU.mult)
                    dd = p_cf.tile(SH, F32, tag="dd")  # u0 - c
                    nc.vector.tensor_sub(dd, u0g, cc)
                    ee = p_cf.tile(SH, F32, tag="ee")
                    nc.vector.tensor_mul(ee, A, dd)
                    uob = p_cf.tile(SH, BF16, tag="uob")
                    nc.vector.tensor_add(uob, ee, cc)
                else:
                    # overlapped quad: one DVE psum->sbuf copy + one fast
                    # reciprocal, then the whole chain on the otherwise
                    # idle GPSIMD (A by repeated squaring: 100 = 64+32+4).
                    qp8 = p_cf.tile(SH8, F32, tag="qp8")
                    nc.vector.tensor_copy(qp8, psq[:, :, :, 0:8])
                    q8 = qp8[:, :, :, 0:4]
                    p8 = qp8[:, :, :, 4:8]
                    rq8m = p_cf.tile(SH8, F32, tag="rq8m")
                    nc.vector.reciprocal_approx_fast(
                        rq8m.rearrange("x c u j -> x (c u) j"),
                        qp8.rearrange("x c u j -> x (c u) j"),
                    )
                    s0m = rq8m[:, :, :, 0:4]
                    G_ = nc.gpsimd
                    a = p_cf.tile(SH, F32, tag="a")
                    G_.tensor_scalar(a, q8, -2.0 * LR, 1.0, ALU.mult, ALU.add)
                    pw = {1: a}
                    for e in (2, 4, 8, 16, 32, 64):
                        t_ = p_cf.tile(SH, F32, tag=f"a{e}")
                        G_.tensor_mul(t_, pw[e // 2], pw[e // 2])
                        pw[e] = t_
                    a96 = p_cf.tile(SH, F32, tag="a96")
                    G_.tensor_mul(a96, pw[64], pw[32])
                    A = p_cf.tile(SH, F32, tag="Am")
                    G_.tensor_mul(A, a96, pw[4])
                    cc = p_cf.tile(SH, F32, tag="ccm")
                    nc.vector.scalar_tensor_tensor(
                        cc, p8, -0.5, s0m, ALU.mult, ALU.mult
                    )
                    dd = p_cf.tile(SH, F32, tag="ddm")
                    G_.tensor_sub(dd, u0g, cc)
                    ee = p_cf.tile(SH, F32, tag="eem")
                    G_.tensor_mul(ee, A, dd)
                    uob = p_cf.tile(SH, BF16, tag="uobm")
                    G_.tensor_add(uob, ee, cc)
                if last:
                    nc.sync.dma_start(out=uo[G, :, 0:2], in_=uob[:, 0:2])
                    nc.scalar.dma_start(out=uo[G, :, 2:4], in_=uob[:, 2:4])
                else:
                    nc.sync.dma_start(out=uo[G], in_=uob)

            pend = None
            for g in range(NT // 2):
                ts = (2 * g, 2 * g + 1)
                obsb = {t: obst[t] for t in ts}
                if g % 2 == 0:
                    z3q = pp_q.tile([64, BT], F32, name="z3q", tag="z3q", bufs=1)

                # layer 1: psum = z1 (scales folded on host).  Pair 0 runs
                # tile-major (t0's four m-chunks while obs1 is still in
                # flight, DVE-only drains since the ACT tables are still
                # loading); later pairs share each stationary across the
                # pair.  psum groups span 2 banks; drains are half-split.
                y1 = {
                    t: p_y1.tile(
                        [128, 4, HID], FP8, name=f"y1_{t % 2}", tag=f"y1_{t % 2}"
                    )
                    for t in ts
                }
                if g == 0:
                    # pair 0 runs fully tile-major (t0's L1 AND L2 while
                    # obs1 is still in flight; t0 L1 drains DVE-only since
                    # the ACT tables are still loading) so the PE never
                    # idles on the second obs tile and HAM warms through
                    y2 = {
                        t: p_y2.tile(
                            [128, 4, HID], FP8, name=f"y2_{t % 2}", tag=f"y2_{t % 2}"
                        )
                        for t in ts
                    }
                    for i, t in enumerate(ts):
                        for m in range(4):
                            ps1 = pp_y.tile([128, BT], F32, name="ps1", tag="y")
                            nc.tensor.matmul(
                                ps1,
                                w1s[:, 0:2, m * 128 : (m + 1) * 128],
                                obsb[t],
                                start=True,
                                stop=True,
                                perf_mode=DR,
                            )
                            drain1(
                                y1[t][:, m, :], ps1, b1s, m,
                                on_act=(i == 1 and (m + i) % 2 == 0),
                            )
                        for m in range(4):
                            ps2 = pp_y.tile([128, BT], F32, name="ps2", tag="y")
                            for kc in range(2):
                                nc.tensor.matmul(
                                    ps2,
                                    w2s[:, 2 * kc : 2 * kc + 2, m * 128 : (m + 1) * 128],
                                    y1[t][:, 2 * kc : 2 * kc + 2, :],
                                    start=(kc == 0),
                                    stop=(kc == 1),
                                    perf_mode=DR,
                                )
                            drain1(y2[t][:, m, :], ps2, b2s, m, (m + i) % 2 == 1)
                    for i, t in enumerate(ts):
                        u = i
                        for kc in range(2):
                            nc.tensor.matmul(
                                z3q[0:64, :],
                                w3s[:, 2 * kc : 2 * kc + 2, 64 * u : 64 * u + 64],
                                y2[t][:, 2 * kc : 2 * kc + 2, :],
                                start=(u == 0 and kc == 0),
                                stop=False,
                                perf_mode=DR,
                            )
                    continue
                else:
                    for m in range(4):
                        ps1 = {
                            t: pp_y.tile([128, BT], F32, name="ps1", tag="y")
                            for t in ts
                        }
                        for t in ts:
                            nc.tensor.matmul(
                                ps1[t],
                                w1s[:, 0:2, m * 128 : (m + 1) * 128],
                                obsb[t],
                                start=True,
                                stop=True,
                                perf_mode=DR,
                            )
                        for i, t in enumerate(ts):
                            drain1(y1[t][:, m, :], ps1[t], b1s, m, (m + i) % 2 == 0)

                if pend is not None:
                    tail(*pend)
                    pend = None

                # layer 2: psum = 64*z2; drain y2' = relu(psum + 64*b2) = 64*y2
                y2 = {
                    t: p_y2.tile(
                        [128, 4, HID], FP8, name=f"y2_{t % 2}", tag=f"y2_{t % 2}"
                    )
                    for t in ts
                }
                for m in range(4):
                    ps2 = {
                        t: pp_y.tile([128, BT], F32, name="ps2", tag="y") for t in ts
                    }
                    for kc in range(2):
                        for t in ts:
                            nc.tensor.matmul(
                                ps2[t],
                                w2s[:, 2 * kc : 2 * kc + 2, m * 128 : (m + 1) * 128],
                                y1[t][:, 2 * kc : 2 * kc + 2, :],
                                start=(kc == 0),
                                stop=(kc == 1),
                                perf_mode=DR,
                            )
                    for i, t in enumerate(ts):
                        drain1(y2[t][:, m, :], ps2[t], b2s, m, (m + i) % 2 == 1)

                # layer 3: psum = 1024*z3 for both tiles in ONE bank (tile u
                # of the quad -> psum rows 16u:16u+16 -> concurrent
                # col-groups), then fused bias+sigmoid on ACT into a
                # quad-merged qpT [64, 512]
                # kc-outer: the two kc0 matmuls (which only need the m0/m1
                # y2 drains) run while the m2/m3 drains finish, so kc1 never
                # waits at the pair boundary
                for kc in range(2):
                    for i, t in enumerate(ts):
                        u = 2 * (g % 2) + i
                        nc.tensor.matmul(
                            z3q[0:64, :],
                            w3s[:, 2 * kc : 2 * kc + 2, 64 * u : 64 * u + 64],
                            y2[t][:, 2 * kc : 2 * kc + 2, :],
                            start=(u == 0 and kc == 0),
                            stop=(u == 3 and kc == 1),
                            perf_mode=DR,
                        )
                if g % 2 == 1:
                    lastq = g // 2 == NT // 4 - 1
                    qpT = p_qp.tile(
                        [64, BT], F32 if lastq else BF16,
                        tag="qpTf" if lastq else "qpTb",
                    )
                    nc.scalar.activation(
                        out=qpT,
                        in_=z3q,
                        func=AF.Sigmoid,
                        bias=b3s[:, 0:1],
                        scale=1.0 / 1024.0,
                    )
                    pend = (g // 2, qpT, u0b[g // 2])
            tail(*pend)
    nc.finalize()
    return nc


def _get_nc(zero_bias):
    key = ("nc", zero_bias)
    if key not in _CACHE:
        _CACHE[key] = _build_nc(zero_bias)
    return _CACHE[key]


FP8NP = ml_dtypes.float8_e4m3  # TRN float8e4: bias 7, max normal +-240


def _to_fp8(x):
    return np.ascontiguousarray(np.clip(x, -240.0, 240.0)).astype(FP8NP)


def kernel(obs, x_init, u_init, W1, b1, W2, b2, W3, b3):
    obs = np.asarray(obs, dtype=np.float32)
    u_init = np.ascontiguousarray(np.asarray(u_init, dtype=np.float32))
    W1 = np.asarray(W1, dtype=np.float32)
    W2 = np.asarray(W2, dtype=np.float32)
    W3 = np.asarray(W3, dtype=np.float32)
    b1 = np.asarray(b1, dtype=np.float32)
    b2 = np.asarray(b2, dtype=np.float32)
    b3 = np.asarray(b3, dtype=np.float32)

    # weights with fp8 scale folding (see module docstring)
    w1h = _to_fp8((8.0 * W1).reshape(2, 128, HID).transpose(1, 0, 2))
    w2h = _to_fp8((64.0 * W2).reshape(4, 128, HID).transpose(1, 0, 2))
    # four 64-wide W3 variants: tile u of a quad -> psum rows 16u:16u+16
    w3u = np.zeros((HID, 256), dtype=np.float32)
    for u in range(4):
        w3u[:, 64 * u + 16 * u : 64 * u + 16 * u + 4] = 16.0 * W3[:, 12:16]
        w3u[:, 64 * u + 16 * u + 4 : 64 * u + 16 * u + 8] = 16.0 * W3[:, 28:32]
    w3h = _to_fp8(w3u.reshape(4, 128, 256).transpose(1, 0, 2))
    b1p = np.ascontiguousarray(b1.reshape(4, 128).T)
    b2p = np.ascontiguousarray(64.0 * b2.reshape(4, 128).T)
    b3p = np.zeros((64, 1), dtype=np.float32)
    for u in range(4):
        b3p[16 * u : 16 * u + 4, 0] = b3[12:16]
        b3p[16 * u + 4 : 16 * u + 8, 0] = b3[28:32]
    idp = np.eye(64, dtype=np.float32)

    zero_bias = bool(np.all(b1 == 0.0) and np.all(b2 == 0.0))
    nc = _get_nc(zero_bias)
    in_maps = []
    for i in range(NCORES):
        oc = obs[i * BPC : (i + 1) * BPC]  # [4096, 256]
        # [t, p, kc, n] = obs[t*512+n, kc*128+p] / 8
        obsT = _to_fp8(oc.reshape(NT, BT, 2, 128).transpose(0, 3, 2, 1) / 8.0)
        # u0[G, p, c, u, j] = u_init[(4G+u)*512 + c*128 + p, j]
        u0c = u_init[i * BPC : (i + 1) * BPC].reshape(NT // 4, 4, 4, 128, 4)
        u0c = np.ascontiguousarray(u0c.transpose(0, 3, 2, 1, 4))
        in_maps.append(
            {
                "obsT": obsT,
                "u0": u0c,
                "w1": w1h,
                "w2": w2h,
                "w3": w3h,
                "b1": b1p,
                "b2": b2p,
                "b3": b3p,
                "idm": idp,
            }
        )
    import os

    kw = {}
    if os.environ.get("BASSK_TRACE"):
        kw = {"trace": True, "tmpdir": os.environ.get("BASSK_TRACE_DIR") or None}
    res = run_bass_kernel_spmd(nc, in_maps, list(range(NCORES)), **kw)
    _CACHE["last_result"] = res
    outs = []
    for i in range(NCORES):
        arr = np.asarray(res.results[i]["uo"]).astype(np.float32)  # [g,p,c,u,j]
        outs.append(arr.transpose(0, 3, 2, 1, 4).reshape(BPC, 4))
    return np.concatenate(outs, axis=0).astype(np.float32)


# revision 45
# speedup vs baseline: 1.2649x; 1.2649x over previous
"""Trainium2 Bass kernel for nn_MPCActor: MLP (256->512->512->8-useful-cols,
relu/relu/sigmoid) followed by the closed-form equivalent of 100 SGD steps on
u (u_N = A*u0 + (1-A)*c, A = (1-2*lr*q)^100, c = -p/(2q); x_init never
enters the u-gradient).

Data parallel over 8 NeuronCores: batch 32768 -> 4096 rows per core, weights
replicated, batch tiles processed in pairs. All L1/L2 matmuls run in
fp8(e4m3) with perf_mode=DoubleRow (K=256 contracted per instruction, 2x
FLOPs at the PE's 1 col/cycle streaming rate). The obs transpose, weight
layouts, fp8 casts and scale folding are all done on the host:
  obsT = obs.T/8 (fp8)      W1h = 8*W1 (fp8)   -> psum1 = z1 exactly
  y1   = relu(z1+b1) (fp8)  W2h = 64*W2 (fp8)  -> psum2 = 64*z2
  y2'  = relu(psum2+64*b2) = 64*y2 (fp8)
  W3h  = 16*W3[:, useful 8 cols]               -> psum3 = 1024*z3
  qp   = sigmoid(psum3/1024 + b3)              (one fused ACT op per QUAD)

Startup is DMA-latency-tuned (each HWDGE ring delivers its first ~128KB
about 2us after the billing clock starts, then ~1.9us per DMA): obs tiles
ride the SP ring in consumption order with the small loads slotted into
slack, w1+obs1 ride the ACT ring ahead of the activation-table loads, w2
halves ride GPSIMD SWDGE as a third ring, and a burst of junk matmuls
(uninitialized-value reads into an overwritten psum bank) keeps the PE
HAM-busy through the first-DMA latency so real matmuls run at 2.4GHz.
Pair 0 runs fully tile-major (t0's L1+L2 while obs1 is in flight, t0's L1
drains DVE-only while the ACT tables load).  PSUM is 8 banks: 6 rotating
single-bank y-groups (one drain frees a bank ~0.56us after its matmul,
alternating ACT relu / DVE max) + the quad z3 bank + the transpose bank.
All FOUR tiles of two consecutive pairs (a quad) share one L3 accumulation
group (four 64-wide column-shifted W3 variants -> one [64,512] bank), so a
single sigmoid + four [64,128] PE transposes serve four batch tiles.
Closed form: c = -p/(2q) and
  A = (1-.02q)^100 = exp(-(2q+.02q^2))*(1+O(3e-4)) = sigmoid(-w)/sigmoid(w)
  where w+50 = (0.1414214*q + 7.0710678)^2  (one ACT Square, biases from
  SBUF consts; both sigmoids hit the already-loaded sigmoid table set).
Reciprocals use the ~5x-faster 18-bit reciprocal_approx_fast custom-DVE op.
The overlapped (non-final) quad runs its chain on the otherwise-idle GPSIMD
(A by repeated squaring, 100 = 64+32+4); the final quad is latency-split
across ACT (square+sigmoids straight from psum) and DVE (fast recips +
finish), and the output stores are bf16 (~1e-3 extra rel err,
well under the 2e-2 gate) split across both HWDGE rings.  The walrus-emitted
epilogue (~8.6us: queue drains, barriers, and a serial clear of all 256
semaphores) and the ~2us first-DMA latency are fixed costs this kernel
cannot remove.
"""

import numpy as np
import ml_dtypes

import concourse.mybir as mybir
import concourse.tile as tile
from concourse import bacc
from concourse.bass_utils import run_bass_kernel_spmd

NCORES = 8
BATCH = 32768
BPC = BATCH // NCORES  # 4096 rows per core
OBS = 256
HID = 512
NQP = 16  # q_u (4) + p_u (4) + 8 zero-pad cols (step%16==0 for DoubleRow)
BT = 512  # batch tile (matmul moving free dim)
NT = BPC // BT  # 8 batch tiles per core
LR = 0.01
NWARM = 14  # junk matmuls (N=256) covering first-DMA latency + HAM warm-up
F32 = mybir.dt.float32
FP8 = mybir.dt.float8e4
BF16 = mybir.dt.bfloat16
FP16 = mybir.dt.float16
DR = mybir.MatmulPerfMode.DoubleRow

_CACHE = {}


def _build_nc(zero_bias):
    nc = bacc.Bacc(
        trn_type="TRN2", target_bir_lowering=False, debug=False, num_devices=NCORES
    )
    obsT = nc.declare_dram_parameter("obsT", [NT, 128, 2, BT], FP8, isOutput=False).ap()
    u0 = nc.declare_dram_parameter("u0", [NT // 4, 128, 4, 4, 4], F32, isOutput=False).ap()
    w1 = nc.declare_dram_parameter("w1", [128, 2, HID], FP8, isOutput=False).ap()
    w2 = nc.declare_dram_parameter("w2", [128, 4, HID], FP8, isOutput=False).ap()
    w3 = nc.declare_dram_parameter("w3", [128, 4, 256], FP8, isOutput=False).ap()
    b1 = nc.declare_dram_parameter("b1", [128, 4], F32, isOutput=False).ap()
    b2 = nc.declare_dram_parameter("b2", [128, 4], F32, isOutput=False).ap()
    b3 = nc.declare_dram_parameter("b3", [64, 1], F32, isOutput=False).ap()
    idm = nc.declare_dram_parameter("idm", [64, 64], F32, isOutput=False).ap()
    uo = nc.declare_dram_parameter("uo", [NT // 4, 128, 4, 4, 4], BF16, isOutput=True).ap()

    AF = mybir.ActivationFunctionType
    ALU = mybir.AluOpType

    with tile.TileContext(nc) as tc:
        from contextlib import ExitStack

        with ExitStack() as ctx:
            singles = ctx.enter_context(tc.tile_pool(name="singles", bufs=1))
            p_y1 = ctx.enter_context(tc.tile_pool(name="y1", bufs=2))
            p_y2 = ctx.enter_context(tc.tile_pool(name="y2", bufs=2))
            p_qp = ctx.enter_context(tc.tile_pool(name="qp", bufs=2))
            p_cf = ctx.enter_context(tc.tile_pool(name="cf", bufs=2))
            # PSUM budget is 8 banks: y 6x1 + z3 quad-group 1 + psq 1
            pp_y = ctx.enter_context(tc.tile_pool(name="ppy", bufs=6, space="PSUM"))
            pp_q = ctx.enter_context(tc.tile_pool(name="ppq", bufs=1, space="PSUM"))

            # ---- junk tile for PE warm-up (memset first on the otherwise
            # idle GPSIMD so the Tensor queue barely waits) ----
            junk = singles.tile([128, 2, 256], FP8)
            nc.gpsimd.memset(junk, 1.0)
            # per-partition constants for the tail's ACT bias operands
            cb_sq = singles.tile([128, 1], F32, name="cb_sq")
            nc.gpsimd.memset(cb_sq, 7.0710678)
            cb_p50 = singles.tile([128, 1], F32, name="cb_p50")
            nc.gpsimd.memset(cb_p50, 50.0)
            cb_m50 = singles.tile([128, 1], F32, name="cb_m50")
            nc.gpsimd.memset(cb_m50, -50.0)

            # ---- input DMAs, latency-ordered.  SP ring: the eight obs
            # tiles in consumption order with the small loads (b3/idm/u0)
            # slotted into the slack; ACT ring: w1, then the two w2 halves,
            # then (behind the act-table loads) the late obs are NOT here --
            # all obs stay on SP so the table loads never delay them. ----
            obst = [singles.tile([128, 2, BT], FP8, name=f"obs{t}") for t in range(NT)]
            w1s = singles.tile([128, 2, HID], FP8)
            w2s = singles.tile([128, 4, HID], FP8)
            w3s = singles.tile([128, 4, 256], FP8)
            b3s = singles.tile([64, 1], F32)
            ids = singles.tile([64, 64], F32)

            nc.sync.dma_start(out=obst[0], in_=obsT[0])
            nc.scalar.dma_start(out=w1s, in_=w1)
            nc.scalar.dma_start(out=obst[1], in_=obsT[1])
            # w2 rides GPSIMD SWDGE (third ring; GPSIMD is idle at startup)
            nc.gpsimd.dma_start(out=w2s[:, 0:2, :], in_=w2[:, 0:2, :])
            nc.gpsimd.dma_start(out=w2s[:, 2:4, :], in_=w2[:, 2:4, :])
            nc.sync.dma_start(out=obst[2], in_=obsT[2])
            nc.sync.dma_start(out=obst[3], in_=obsT[3])
            nc.sync.dma_start(out=obst[4], in_=obsT[4])
            nc.sync.dma_start(out=b3s, in_=b3, single_packet=True)
            nc.sync.dma_start(out=ids, in_=idm, single_packet=True)
            u0b = {}
            for G in range(NT // 4):
                u0b[G] = p_cf.tile([128, 4, 4, 4], F32, name=f"u0b{G}", tag=f"u0b{G}")
            nc.sync.dma_start(out=obst[5], in_=obsT[5])
            nc.sync.dma_start(out=u0b[0], in_=u0[0], single_packet=True)
            nc.sync.dma_start(out=obst[6], in_=obsT[6])
            nc.sync.dma_start(out=obst[7], in_=obsT[7])
            nc.sync.dma_start(out=u0b[1], in_=u0[1], single_packet=True)
            if zero_bias:
                b1s = b2s = None
            else:
                b1s = singles.tile([128, 4], F32)
                nc.sync.dma_start(out=b1s, in_=b1)
                b2s = singles.tile([128, 4], F32)
                nc.sync.dma_start(out=b2s, in_=b2)

            # dummy sigmoid on ACT (after the ACT ring's DMA issues) so the
            # sigmoid-capable activation table set (which also contains
            # Relu) loads at startup, off the drain critical path
            dum = singles.tile([128, 16], F32)
            nc.scalar.activation(
                out=dum, in_=junk[:, 0, 0:16], func=AF.Sigmoid, bias=0.0, scale=1.0
            )
            nc.scalar.dma_start(out=w3s, in_=w3)

            # ---- PE warm-up: junk matmuls, one shared stationary ----
            wps = pp_y.tile([128, BT], F32, name="wps", tag="y")
            for i in range(NWARM):
                nc.tensor.matmul(
                    wps[:, 0:256],
                    junk[:, 0:2, 0:128],
                    junk,
                    start=True,
                    stop=True,
                    perf_mode=DR,
                )

            def drain1(dst, src, bias_sb, m, on_act):
                # dst [128, 512] fp8 <- relu(src [128, 512] single-bank psum
                # + bias); one drain per bank keeps the 6-deep rotation
                # smooth and frees each bank ~0.56us after its last matmul.
                b = None if zero_bias else bias_sb[:, m : m + 1]
                if on_act:
                    nc.scalar.activation(
                        out=dst,
                        in_=src,
                        func=AF.Relu,
                        bias=0.0 if zero_bias else b,
                        scale=1.0,
                    )
                elif zero_bias:
                    nc.vector.tensor_scalar(dst, src, 0.0, None, ALU.max)
                else:
                    nc.vector.tensor_scalar(dst, src, b, 0.0, ALU.add, ALU.max)

            def tail(G, qpT, u0g):
                # transpose to batch-major + closed form + store, for quad G
                # (4 batch tiles). Emitted AFTER the next pair's layer-1
                # matmuls so the PE never idles on the sigmoid.
                last = G == NT // 4 - 1
                psq4 = pp_q.tile([128, 4, 64], F32, tag="psq")
                for c in range(4):
                    nc.tensor.transpose(
                        psq4[:, c, :], qpT[:, c * 128 : (c + 1) * 128], ids[:]
                    )
                psq = psq4.rearrange("x c (u j) -> x c u j", u=4)
                q = psq[:, :, :, 0:4]
                p = psq[:, :, :, 4:8]
                SH = [128, 4, 4, 4]
                SH8 = [128, 4, 4, 8]
                if last:
                    # latency-split chain.  ACT (reads psum directly):
                    #   yq = (0.1414q+7.071)^2 = .02q^2+2q+50 = w+50
                    #   s2 = sigmoid(yq-50) = sigmoid(w); s1 = sigmoid(-w)
                    # DVE: fast reciprocals (18-bit, plenty) + finish;
                    # GPSIMD: the c-branch products.
                    yq = p_cf.tile(SH, F32, tag="yq")
                    nc.scalar.activation(
                        out=yq, in_=q, func=AF.Square, bias=cb_sq[:, 0:1],
                        scale=0.14142136,
                    )
                    s2 = p_cf.tile(SH, F32, tag="s2")  # sigmoid(w)
                    nc.scalar.activation(
                        out=s2, in_=yq, func=AF.Sigmoid, bias=cb_m50[:, 0:1],
                        scale=1.0,
                    )
                    s1 = p_cf.tile(SH, F32, tag="s1")  # sigmoid(-w)
                    nc.scalar.activation(
                        out=s1, in_=yq, func=AF.Sigmoid, bias=cb_p50[:, 0:1],
                        scale=-1.0,
                    )
                    p8 = psq[:, :, :, 4:8]
                    rq8 = p_cf.tile(SH, F32, tag="rq8")  # 1/q
                    nc.vector.reciprocal_approx_fast(
                        rq8.rearrange("x c u j -> x (c u) j"),
                        psq4.rearrange("x c (u j) -> x (c u) j", u=4)[:, :, 0:4],
                    )
                    s0 = rq8
                    rr = p_cf.tile(SH, F32, tag="rr")  # 1/sigmoid(w)
                    nc.vector.reciprocal_approx_fast(
                        rr.rearrange("x c u j -> x (c u) j"),
                        s2.rearrange("x c u j -> x (c u) j"),
                    )
                    A = p_cf.tile(SH, F32, tag="A")  # e^-w = s1/s2
                    nc.vector.tensor_mul(A, s1, rr)
                    cc = p_cf.tile(SH, F32, tag="cc")  # -p/(2q)
                    nc.vector.scalar_tensor_tensor(cc, p8, -0.5, s0, ALU.mult, ALU.mult)
                    dd = p_cf.tile(SH, F32, tag="dd")  # u0 - c
                    nc.vector.tensor_sub(dd, u0g, cc)
                    ee = p_cf.tile(SH, F32, tag="ee")
                    nc.vector.tensor_mul(ee, A, dd)
                    uob = p_cf.tile(SH, BF16, tag="uob")
                    nc.vector.tensor_add(uob, ee, cc)
                else:
                    # overlapped quad: one DVE psum->sbuf copy + one fast
                    # reciprocal, then the whole chain on the otherwise
                    # idle GPSIMD (A by repeated squaring: 100 = 64+32+4).
                    qp8 = p_cf.tile(SH8, F32, tag="qp8")
                    nc.vector.tensor_copy(qp8, psq[:, :, :, 0:8])
                    q8 = qp8[:, :, :, 0:4]
                    p8 = qp8[:, :, :, 4:8]
                    rq8m = p_cf.tile(SH8, F32, tag="rq8m")
                    nc.vector.reciprocal_approx_fast(
                        rq8m.rearrange("x c u j -> x (c u) j"),
                        qp8.rearrange("x c u j -> x (c u) j"),
                    )
                    s0m = rq8m[:, :, :, 0:4]
                    G_ = nc.gpsimd
                    a = p_cf.tile(SH, F32, tag="a")
                    G_.tensor_scalar(a, q8, -2.0 * LR, 1.0, ALU.mult, ALU.add)
                    pw = {1: a}
                    for e in (2, 4, 8, 16, 32, 64):
                        t_ = p_cf.tile(SH, F32, tag=f"a{e}")
                        G_.tensor_mul(t_, pw[e // 2], pw[e // 2])
                        pw[e] = t_
                    a96 = p_cf.tile(SH, F32, tag="a96")
                    G_.tensor_mul(a96, pw[64], pw[32])
                    A = p_cf.tile(SH, F32, tag="Am")
                    G_.tensor_mul(A, a96, pw[4])
                    cc = p_cf.tile(SH, F32, tag="ccm")
                    nc.vector.scalar_tensor_tensor(
                        cc, p8, -0.5, s0m, ALU.mult, ALU.mult
                    )
                    dd = p_cf.tile(SH, F32, tag="ddm")
                    G_.tensor_sub(dd, u0g, cc)
                    ee = p_cf.tile(SH, F32, tag="eem")
                    G_.tensor_mul(ee, A, dd)
                    uob = p_cf.tile(SH, BF16, tag="uobm")
                    G_.tensor_add(uob, ee, cc)
                if last:
                    nc.sync.dma_start(out=uo[G, :, 0:2], in_=uob[:, 0:2])
                    nc.scalar.dma_start(out=uo[G, :, 2:4], in_=uob[:, 2:4])
                else:
                    nc.sync.dma_start(out=uo[G], in_=uob)

            pend = None
            for g in range(NT // 2):
                ts = (2 * g, 2 * g + 1)
                obsb = {t: obst[t] for t in ts}
                if g % 2 == 0:
                    z3q = pp_q.tile([64, BT], F32, name="z3q", tag="z3q", bufs=1)

                # layer 1: psum = z1 (scales folded on host).  Pair 0 runs
                # tile-major (t0's four m-chunks while obs1 is still in
                # flight, DVE-only drains since the ACT tables are still
                # loading); later pairs share each stationary across the
                # pair.  psum groups span 2 banks; drains are half-split.
                y1 = {
                    t: p_y1.tile(
                        [128, 4, HID], FP8, name=f"y1_{t % 2}", tag=f"y1_{t % 2}"
                    )
                    for t in ts
                }
                if g == 0:
                    # pair 0 runs fully tile-major (t0's L1 AND L2 while
                    # obs1 is still in flight; t0 L1 drains DVE-only since
                    # the ACT tables are still loading) so the PE never
                    # idles on the second obs tile and HAM warms through
                    y2 = {
                        t: p_y2.tile(
                            [128, 4, HID], FP8, name=f"y2_{t % 2}", tag=f"y2_{t % 2}"
                        )
                        for t in ts
                    }
                    for i, t in enumerate(ts):
                        for m in range(4):
                            ps1 = pp_y.tile([128, BT], F32, name="ps1", tag="y")
                            nc.tensor.matmul(
                                ps1,
                                w1s[:, 0:2, m * 128 : (m + 1) * 128],
                                obsb[t],
                                start=True,
                                stop=True,
                                perf_mode=DR,
                            )
                            drain1(
                                y1[t][:, m, :], ps1, b1s, m,
                                on_act=(i == 1 and (m + i) % 2 == 0),
                            )
                        for m in range(4):
                            ps2 = pp_y.tile([128, BT], F32, name="ps2", tag="y")
                            for kc in range(2):
                                nc.tensor.matmul(
                                    ps2,
                                    w2s[:, 2 * kc : 2 * kc + 2, m * 128 : (m + 1) * 128],
                                    y1[t][:, 2 * kc : 2 * kc + 2, :],
                                    start=(kc == 0),
                                    stop=(kc == 1),
                                    perf_mode=DR,
                                )
                            drain1(y2[t][:, m, :], ps2, b2s, m, (m + i) % 2 == 1)
                    for i, t in enumerate(ts):
                        u = i
                        for kc in range(2):
                            nc.tensor.matmul(
                                z3q[0:64, :],
                                w3s[:, 2 * kc : 2 * kc + 2, 64 * u : 64 * u + 64],
                                y2[t][:, 2 * kc : 2 * kc + 2, :],
                                start=(u == 0 and kc == 0),
                                stop=False,
                                perf_mode=DR,
                            )
                    continue
                else:
                    for m in range(4):
                        ps1 = {
                            t: pp_y.tile([128, BT], F32, name="ps1", tag="y")
                            for t in ts
                        }
                        for t in ts:
                            nc.tensor.matmul(
                                ps1[t],
                                w1s[:, 0:2, m * 128 : (m + 1) * 128],
                                obsb[t],
                                start=True,
                                stop=True,
                                perf_mode=DR,
                            )
                        for i, t in enumerate(ts):
                            drain1(y1[t][:, m, :], ps1[t], b1s, m, (m + i) % 2 == 0)

                if pend is not None:
                    tail(*pend)
                    pend = None

                # layer 2: psum = 64*z2; drain y2' = relu(psum + 64*b2) = 64*y2
                y2 = {
                    t: p_y2.tile(
                        [128, 4, HID], FP8, name=f"y2_{t % 2}", tag=f"y2_{t % 2}"
                    )
                    for t in ts
                }
                for m in range(4):
                    ps2 = {
                        t: pp_y.tile([128, BT], F32, name="ps2", tag="y") for t in ts
                    }
                    for kc in range(2):
                        for t in ts:
                            nc.tensor.matmul(
                                ps2[t],
                                w2s[:, 2 * kc : 2 * kc + 2, m * 128 : (m + 1) * 128],
                                y1[t][:, 2 * kc : 2 * kc + 2, :],
                                start=(kc == 0),
                                stop=(kc == 1),
                                perf_mode=DR,
                            )
                    for i, t in enumerate(ts):
                        drain1(y2[t][:, m, :], ps2[t], b2s, m, (m + i) % 2 == 1)

                # layer 3: psum = 1024*z3 for both tiles in ONE bank (tile u
                # of the quad -> psum rows 16u:16u+16 -> concurrent
                # col-groups), then fused bias+sigmoid on ACT into a
                # quad-merged qpT [64, 512]
                # kc-outer: the two kc0 matmuls (which only need the m0/m1
                # y2 drains) run while the m2/m3 drains finish, so kc1 never
                # waits at the pair boundary
                for kc in range(2):
                    for i, t in enumerate(ts):
                        u = 2 * (g % 2) + i
                        nc.tensor.matmul(
                            z3q[0:64, :],
                            w3s[:, 2 * kc : 2 * kc + 2, 64 * u : 64 * u + 64],
                            y2[t][:, 2 * kc : 2 * kc + 2, :],
                            start=(u == 0 and kc == 0),
                            stop=(u == 3 and kc == 1),
                            perf_mode=DR,
                        )
                if g % 2 == 1:
                    qpT = p_qp.tile([64, BT], F32, tag="qpT")
                    nc.scalar.activation(
                        out=qpT,
                        in_=z3q,
                        func=AF.Sigmoid,
                        bias=b3s[:, 0:1],
                        scale=1.0 / 1024.0,
                    )
                    pend = (g // 2, qpT, u0b[g // 2])
            tail(*pend)
    nc.finalize()
    return nc


def _get_nc(zero_bias):
    key = ("nc", zero_bias)
    if key not in _CACHE:
        _CACHE[key] = _build_nc(zero_bias)
    return _CACHE[key]


FP8NP = ml_dtypes.float8_e4m3  # TRN float8e4: bias 7, max normal +-240


def _to_fp8(x):
    return np.ascontiguousarray(np.clip(x, -240.0, 240.0)).astype(FP8NP)


def kernel(obs, x_init, u_init, W1, b1, W2, b2, W3, b3):
    obs = np.asarray(obs, dtype=np.float32)
    u_init = np.ascontiguousarray(np.asarray(u_init, dtype=np.float32))
    W1 = np.asarray(W1, dtype=np.float32)
    W2 = np.asarray(W2, dtype=np.float32)
    W3 = np.asarray(W3, dtype=np.float32)
    b1 = np.asarray(b1, dtype=np.float32)
    b2 = np.asarray(b2, dtype=np.float32)
    b3 = np.asarray(b3, dtype=np.float32)

    # weights with fp8 scale folding (see module docstring)
    w1h = _to_fp8((8.0 * W1).reshape(2, 128, HID).transpose(1, 0, 2))
    w2h = _to_fp8((64.0 * W2).reshape(4, 128, HID).transpose(1, 0, 2))
    # four 64-wide W3 variants: tile u of a quad -> psum rows 16u:16u+16
    w3u = np.zeros((HID, 256), dtype=np.float32)
    for u in range(4):
        w3u[:, 64 * u + 16 * u : 64 * u + 16 * u + 4] = 16.0 * W3[:, 12:16]
        w3u[:, 64 * u + 16 * u + 4 : 64 * u + 16 * u + 8] = 16.0 * W3[:, 28:32]
    w3h = _to_fp8(w3u.reshape(4, 128, 256).transpose(1, 0, 2))
    b1p = np.ascontiguousarray(b1.reshape(4, 128).T)
    b2p = np.ascontiguousarray(64.0 * b2.reshape(4, 128).T)
    b3p = np.zeros((64, 1), dtype=np.float32)
    for u in range(4):
        b3p[16 * u : 16 * u + 4, 0] = b3[12:16]
        b3p[16 * u + 4 : 16 * u + 8, 0] = b3[28:32]
    idp = np.eye(64, dtype=np.float32)

    zero_bias = bool(np.all(b1 == 0.0) and np.all(b2 == 0.0))
    nc = _get_nc(zero_bias)
    in_maps = []
    for i in range(NCORES):
        oc = obs[i * BPC : (i + 1) * BPC]  # [4096, 256]
        # [t, p, kc, n] = obs[t*512+n, kc*128+p] / 8
        obsT = _to_fp8(oc.reshape(NT, BT, 2, 128).transpose(0, 3, 2, 1) / 8.0)
        # u0[G, p, c, u, j] = u_init[(4G+u)*512 + c*128 + p, j]
        u0c = u_init[i * BPC : (i + 1) * BPC].reshape(NT // 4, 4, 4, 128, 4)
        u0c = np.ascontiguousarray(u0c.transpose(0, 3, 2, 1, 4))
        in_maps.append(
            {
                "obsT": obsT,
                "u0": u0c,
                "w1": w1h,
                "w2": w2h,
                "w3": w3h,
                "b1": b1p,
                "b2": b2p,
                "b3": b3p,
                "idm": idp,
            }
        )
    import os

    kw = {}
    if os.environ.get("BASSK_TRACE"):
        kw = {"trace": True, "tmpdir": os.environ.get("BASSK_TRACE_DIR") or None}
    res = run_bass_kernel_spmd(nc, in_maps, list(range(NCORES)), **kw)
    _CACHE["last_result"] = res
    outs = []
    for i in range(NCORES):
        arr = np.asarray(res.results[i]["uo"]).astype(np.float32)  # [g,p,c,u,j]
        outs.append(arr.transpose(0, 3, 2, 1, 4).reshape(BPC, 4))
    return np.concatenate(outs, axis=0).astype(np.float32)


# revision 46
# speedup vs baseline: 1.2857x; 1.0164x over previous
"""Trainium2 Bass kernel for nn_MPCActor: MLP (256->512->512->8-useful-cols,
relu/relu/sigmoid) followed by the closed-form equivalent of 100 SGD steps on
u (u_N = A*u0 + (1-A)*c, A = (1-2*lr*q)^100, c = -p/(2q); x_init never
enters the u-gradient).

Data parallel over 8 NeuronCores: batch 32768 -> 4096 rows per core, weights
replicated, batch tiles processed in pairs. All L1/L2 matmuls run in
fp8(e4m3) with perf_mode=DoubleRow (K=256 contracted per instruction, 2x
FLOPs at the PE's 1 col/cycle streaming rate). The obs transpose, weight
layouts, fp8 casts and scale folding are all done on the host:
  obsT = obs.T/8 (fp8)      W1h = 8*W1 (fp8)   -> psum1 = z1 exactly
  y1   = relu(z1+b1) (fp8)  W2h = 64*W2 (fp8)  -> psum2 = 64*z2
  y2'  = relu(psum2+64*b2) = 64*y2 (fp8)
  W3h  = 16*W3[:, useful 8 cols]               -> psum3 = 1024*z3
  qp   = sigmoid(psum3/1024 + b3)              (one fused ACT op per QUAD)

Startup is DMA-latency-tuned (each HWDGE ring delivers its first ~128KB
about 2us after the billing clock starts, then ~1.9us per DMA): obs tiles
ride the SP ring in consumption order with the small loads slotted into
slack, w1+obs1 ride the ACT ring ahead of the activation-table loads, w2
halves ride GPSIMD SWDGE as a third ring, and a burst of junk matmuls
(uninitialized-value reads into an overwritten psum bank) keeps the PE
HAM-busy through the first-DMA latency so real matmuls run at 2.4GHz.
Pair 0 runs fully tile-major (t0's L1+L2 while obs1 is in flight, t0's L1
drains DVE-only while the ACT tables load).  PSUM is 8 banks: 6 rotating
single-bank y-groups (one drain frees a bank ~0.56us after its matmul,
alternating ACT relu / DVE max) + the quad z3 bank + the transpose bank.
All FOUR tiles of two consecutive pairs (a quad) share one L3 accumulation
group (four 64-wide column-shifted W3 variants -> one [64,512] bank), so a
single sigmoid + four [64,128] PE transposes serve four batch tiles.
Closed form: c = -p/(2q) and
  A = (1-.02q)^100 = exp(-(2q+.02q^2))*(1+O(3e-4)) = sigmoid(-w)/sigmoid(w)
  where w+50 = (0.1414214*q + 7.0710678)^2  (one ACT Square, biases from
  SBUF consts; both sigmoids hit the already-loaded sigmoid table set).
Reciprocals use the ~5x-faster 18-bit reciprocal_approx_fast custom-DVE op.
The overlapped (non-final) quad runs its chain on the otherwise-idle GPSIMD
(A by repeated squaring, 100 = 64+32+4); the final quad is latency-split
across ACT (square+sigmoids straight from psum) and DVE (fast recips +
finish), and the output stores are bf16 (~1e-3 extra rel err,
well under the 2e-2 gate) split across both HWDGE rings.  The walrus-emitted
epilogue (~8.6us: queue drains, barriers, and a serial clear of all 256
semaphores) and the ~2us first-DMA latency are fixed costs this kernel
cannot remove.
"""

import numpy as np
import ml_dtypes

import concourse.mybir as mybir
import concourse.tile as tile
from concourse import bacc
from concourse.bass_utils import run_bass_kernel_spmd

NCORES = 8
BATCH = 32768
BPC = BATCH // NCORES  # 4096 rows per core
OBS = 256
HID = 512
NQP = 16  # q_u (4) + p_u (4) + 8 zero-pad cols (step%16==0 for DoubleRow)
BT = 512  # batch tile (matmul moving free dim)
NT = BPC // BT  # 8 batch tiles per core
LR = 0.01
NWARM = 14  # junk matmuls (N=256) covering first-DMA latency + HAM warm-up
F32 = mybir.dt.float32
FP8 = mybir.dt.float8e4
BF16 = mybir.dt.bfloat16
FP16 = mybir.dt.float16
DR = mybir.MatmulPerfMode.DoubleRow

_CACHE = {}


def _build_nc(zero_bias):
    nc = bacc.Bacc(
        trn_type="TRN2", target_bir_lowering=False, debug=False, num_devices=NCORES
    )
    obsT = nc.declare_dram_parameter("obsT", [NT, 128, 2, BT], FP8, isOutput=False).ap()
    u0 = nc.declare_dram_parameter("u0", [NT // 4, 128, 4, 4, 4], F32, isOutput=False).ap()
    w1 = nc.declare_dram_parameter("w1", [128, 2, HID], FP8, isOutput=False).ap()
    w2 = nc.declare_dram_parameter("w2", [128, 4, HID], FP8, isOutput=False).ap()
    w3 = nc.declare_dram_parameter("w3", [128, 4, 256], FP8, isOutput=False).ap()
    b1 = nc.declare_dram_parameter("b1", [128, 4], F32, isOutput=False).ap()
    b2 = nc.declare_dram_parameter("b2", [128, 4], F32, isOutput=False).ap()
    b3 = nc.declare_dram_parameter("b3", [64, 1], F32, isOutput=False).ap()
    idm = nc.declare_dram_parameter("idm", [64, 64], F32, isOutput=False).ap()
    uo = nc.declare_dram_parameter("uo", [NT // 4, 128, 4, 4, 4], BF16, isOutput=True).ap()

    AF = mybir.ActivationFunctionType
    ALU = mybir.AluOpType

    with tile.TileContext(nc) as tc:
        from contextlib import ExitStack

        with ExitStack() as ctx:
            singles = ctx.enter_context(tc.tile_pool(name="singles", bufs=1))
            p_y1 = ctx.enter_context(tc.tile_pool(name="y1", bufs=3))
            p_y2 = ctx.enter_context(tc.tile_pool(name="y2", bufs=3))
            p_qp = ctx.enter_context(tc.tile_pool(name="qp", bufs=2))
            p_cf = ctx.enter_context(tc.tile_pool(name="cf", bufs=2))
            # PSUM budget is 8 banks: y 6x1 + z3 quad-group 1 + psq 1
            pp_y = ctx.enter_context(tc.tile_pool(name="ppy", bufs=6, space="PSUM"))
            pp_q = ctx.enter_context(tc.tile_pool(name="ppq", bufs=1, space="PSUM"))

            # ---- junk tile for PE warm-up (memset first on the otherwise
            # idle GPSIMD so the Tensor queue barely waits) ----
            junk = singles.tile([128, 2, 256], FP8)
            nc.gpsimd.memset(junk, 1.0)
            # per-partition constants for the tail's ACT bias operands
            cb_sq = singles.tile([128, 1], F32, name="cb_sq")
            nc.gpsimd.memset(cb_sq, 7.0710678)
            cb_p50 = singles.tile([128, 1], F32, name="cb_p50")
            nc.gpsimd.memset(cb_p50, 50.0)
            cb_m50 = singles.tile([128, 1], F32, name="cb_m50")
            nc.gpsimd.memset(cb_m50, -50.0)

            # ---- input DMAs, latency-ordered.  SP ring: the eight obs
            # tiles in consumption order with the small loads (b3/idm/u0)
            # slotted into the slack; ACT ring: w1, then the two w2 halves,
            # then (behind the act-table loads) the late obs are NOT here --
            # all obs stay on SP so the table loads never delay them. ----
            obst = [singles.tile([128, 2, BT], FP8, name=f"obs{t}") for t in range(NT)]
            w1s = singles.tile([128, 2, HID], FP8)
            w2s = singles.tile([128, 4, HID], FP8)
            w3s = singles.tile([128, 4, 256], FP8)
            b3s = singles.tile([64, 1], F32)
            ids = singles.tile([64, 64], F32)

            nc.sync.dma_start(out=obst[0], in_=obsT[0])
            nc.scalar.dma_start(out=w1s, in_=w1)
            nc.scalar.dma_start(out=obst[1], in_=obsT[1])
            # w2 rides GPSIMD SWDGE (third ring; GPSIMD is idle at startup)
            nc.gpsimd.dma_start(out=w2s[:, 0:2, :], in_=w2[:, 0:2, :])
            nc.gpsimd.dma_start(out=w2s[:, 2:4, :], in_=w2[:, 2:4, :])
            nc.sync.dma_start(out=obst[2], in_=obsT[2])
            nc.sync.dma_start(out=obst[3], in_=obsT[3])
            nc.sync.dma_start(out=obst[4], in_=obsT[4])
            nc.sync.dma_start(out=b3s, in_=b3, single_packet=True)
            nc.sync.dma_start(out=ids, in_=idm, single_packet=True)
            u0b = {}
            for G in range(NT // 4):
                u0b[G] = p_cf.tile([128, 4, 4, 4], F32, name=f"u0b{G}", tag=f"u0b{G}")
            nc.sync.dma_start(out=obst[5], in_=obsT[5])
            nc.sync.dma_start(out=u0b[0], in_=u0[0], single_packet=True)
            nc.sync.dma_start(out=obst[6], in_=obsT[6])
            nc.sync.dma_start(out=obst[7], in_=obsT[7])
            nc.sync.dma_start(out=u0b[1], in_=u0[1], single_packet=True)
            if zero_bias:
                b1s = b2s = None
            else:
                b1s = singles.tile([128, 4], F32)
                nc.sync.dma_start(out=b1s, in_=b1)
                b2s = singles.tile([128, 4], F32)
                nc.sync.dma_start(out=b2s, in_=b2)

            # dummy sigmoid on ACT (after the ACT ring's DMA issues) so the
            # sigmoid-capable activation table set (which also contains
            # Relu) loads at startup, off the drain critical path
            dum = singles.tile([128, 16], F32)
            nc.scalar.activation(
                out=dum, in_=junk[:, 0, 0:16], func=AF.Sigmoid, bias=0.0, scale=1.0
            )
            nc.scalar.dma_start(out=w3s, in_=w3)

            # ---- PE warm-up: junk matmuls, one shared stationary ----
            wps = pp_y.tile([128, BT], F32, name="wps", tag="y")
            for i in range(NWARM):
                nc.tensor.matmul(
                    wps[:, 0:256],
                    junk[:, 0:2, 0:128],
                    junk,
                    start=True,
                    stop=True,
                    perf_mode=DR,
                )

            def drain1(dst, src, bias_sb, m, on_act):
                # dst [128, 512] fp8 <- relu(src [128, 512] single-bank psum
                # + bias); one drain per bank keeps the 6-deep rotation
                # smooth and frees each bank ~0.56us after its last matmul.
                b = None if zero_bias else bias_sb[:, m : m + 1]
                if on_act:
                    nc.scalar.activation(
                        out=dst,
                        in_=src,
                        func=AF.Relu,
                        bias=0.0 if zero_bias else b,
                        scale=1.0,
                    )
                elif zero_bias:
                    nc.vector.tensor_scalar(dst, src, 0.0, None, ALU.max)
                else:
                    nc.vector.tensor_scalar(dst, src, b, 0.0, ALU.add, ALU.max)

            def tail(G, qpT, u0g):
                # transpose to batch-major + closed form + store, for quad G
                # (4 batch tiles). Emitted AFTER the next pair's layer-1
                # matmuls so the PE never idles on the sigmoid.
                last = G == NT // 4 - 1
                psq4 = pp_q.tile([128, 4, 64], F32, tag="psq")
                for c in range(4):
                    nc.tensor.transpose(
                        psq4[:, c, :], qpT[:, c * 128 : (c + 1) * 128], ids[:]
                    )
                psq = psq4.rearrange("x c (u j) -> x c u j", u=4)
                q = psq[:, :, :, 0:4]
                p = psq[:, :, :, 4:8]
                SH = [128, 4, 4, 4]
                SH8 = [128, 4, 4, 8]
                if last:
                    # latency-split chain.  ACT (reads psum directly):
                    #   yq = (0.1414q+7.071)^2 = .02q^2+2q+50 = w+50
                    #   s2 = sigmoid(yq-50) = sigmoid(w); s1 = sigmoid(-w)
                    # DVE: fast reciprocals (18-bit, plenty) + finish;
                    # GPSIMD: the c-branch products.
                    yq = p_cf.tile(SH, F32, tag="yq")
                    nc.scalar.activation(
                        out=yq, in_=q, func=AF.Square, bias=cb_sq[:, 0:1],
                        scale=0.14142136,
                    )
                    s2 = p_cf.tile(SH, F32, tag="s2")  # sigmoid(w)
                    nc.scalar.activation(
                        out=s2, in_=yq, func=AF.Sigmoid, bias=cb_m50[:, 0:1],
                        scale=1.0,
                    )
                    s1 = p_cf.tile(SH, F32, tag="s1")  # sigmoid(-w)
                    nc.scalar.activation(
                        out=s1, in_=yq, func=AF.Sigmoid, bias=cb_p50[:, 0:1],
                        scale=-1.0,
                    )
                    p8 = psq[:, :, :, 4:8]
                    rq8 = p_cf.tile(SH, F32, tag="rq8")  # 1/q
                    nc.vector.reciprocal_approx_fast(
                        rq8.rearrange("x c u j -> x (c u) j"),
                        psq4.rearrange("x c (u j) -> x (c u) j", u=4)[:, :, 0:4],
                    )
                    s0 = rq8
                    rr = p_cf.tile(SH, F32, tag="rr")  # 1/sigmoid(w)
                    nc.vector.reciprocal_approx_fast(
                        rr.rearrange("x c u j -> x (c u) j"),
                        s2.rearrange("x c u j -> x (c u) j"),
                    )
                    A = p_cf.tile(SH, F32, tag="A")  # e^-w = s1/s2
                    nc.vector.tensor_mul(A, s1, rr)
                    cc = p_cf.tile(SH, F32, tag="cc")  # -p/(2q)
                    nc.vector.scalar_tensor_tensor(cc, p8, -0.5, s0, ALU.mult, ALU.mult)
                    dd = p_cf.tile(SH, F32, tag="dd")  # u0 - c
                    nc.vector.tensor_sub(dd, u0g, cc)
                    ee = p_cf.tile(SH, F32, tag="ee")
                    nc.vector.tensor_mul(ee, A, dd)
                    uob = p_cf.tile(SH, BF16, tag="uob")
                    nc.vector.tensor_add(uob, ee, cc)
                else:
                    # overlapped quad: one DVE psum->sbuf copy + one fast
                    # reciprocal, then the whole chain on the otherwise
                    # idle GPSIMD (A by repeated squaring: 100 = 64+32+4).
                    qp8 = p_cf.tile(SH8, F32, tag="qp8")
                    nc.vector.tensor_copy(qp8, psq[:, :, :, 0:8])
                    q8 = qp8[:, :, :, 0:4]
                    p8 = qp8[:, :, :, 4:8]
                    rq8m = p_cf.tile(SH8, F32, tag="rq8m")
                    nc.vector.reciprocal_approx_fast(
                        rq8m.rearrange("x c u j -> x (c u) j"),
                        qp8.rearrange("x c u j -> x (c u) j"),
                    )
                    s0m = rq8m[:, :, :, 0:4]
                    G_ = nc.gpsimd
                    a = p_cf.tile(SH, F32, tag="a")
                    G_.tensor_scalar(a, q8, -2.0 * LR, 1.0, ALU.mult, ALU.add)
                    pw = {1: a}
                    for e in (2, 4, 8, 16, 32, 64):
                        t_ = p_cf.tile(SH, F32, tag=f"a{e}")
                        G_.tensor_mul(t_, pw[e // 2], pw[e // 2])
                        pw[e] = t_
                    a96 = p_cf.tile(SH, F32, tag="a96")
                    G_.tensor_mul(a96, pw[64], pw[32])
                    A = p_cf.tile(SH, F32, tag="Am")
                    G_.tensor_mul(A, a96, pw[4])
                    cc = p_cf.tile(SH, F32, tag="ccm")
                    nc.vector.scalar_tensor_tensor(
                        cc, p8, -0.5, s0m, ALU.mult, ALU.mult
                    )
                    dd = p_cf.tile(SH, F32, tag="ddm")
                    G_.tensor_sub(dd, u0g, cc)
                    ee = p_cf.tile(SH, F32, tag="eem")
                    G_.tensor_mul(ee, A, dd)
                    uob = p_cf.tile(SH, BF16, tag="uobm")
                    G_.tensor_add(uob, ee, cc)
                if last:
                    nc.sync.dma_start(out=uo[G, :, 0:2], in_=uob[:, 0:2])
                    nc.scalar.dma_start(out=uo[G, :, 2:4], in_=uob[:, 2:4])
                else:
                    nc.sync.dma_start(out=uo[G], in_=uob)

            pend = None
            for g in range(NT // 2):
                ts = (2 * g, 2 * g + 1)
                obsb = {t: obst[t] for t in ts}
                if g % 2 == 0:
                    z3q = pp_q.tile([64, BT], F32, name="z3q", tag="z3q", bufs=1)

                # layer 1: psum = z1 (scales folded on host).  Pair 0 runs
                # tile-major (t0's four m-chunks while obs1 is still in
                # flight, DVE-only drains since the ACT tables are still
                # loading); later pairs share each stationary across the
                # pair.  psum groups span 2 banks; drains are half-split.
                y1 = {
                    t: p_y1.tile(
                        [128, 4, HID], FP8, name=f"y1_{t % 2}", tag=f"y1_{t % 2}"
                    )
                    for t in ts
                }
                if g == 0:
                    # pair 0 runs fully tile-major (t0's L1 AND L2 while
                    # obs1 is still in flight; t0 L1 drains DVE-only since
                    # the ACT tables are still loading) so the PE never
                    # idles on the second obs tile and HAM warms through
                    y2 = {
                        t: p_y2.tile(
                            [128, 4, HID], FP8, name=f"y2_{t % 2}", tag=f"y2_{t % 2}"
                        )
                        for t in ts
                    }
                    for i, t in enumerate(ts):
                        for m in range(4):
                            ps1 = pp_y.tile([128, BT], F32, name="ps1", tag="y")
                            nc.tensor.matmul(
                                ps1,
                                w1s[:, 0:2, m * 128 : (m + 1) * 128],
                                obsb[t],
                                start=True,
                                stop=True,
                                perf_mode=DR,
                            )
                            drain1(
                                y1[t][:, m, :], ps1, b1s, m,
                                on_act=(i == 1 and (m + i) % 2 == 0),
                            )
                        for m in range(4):
                            ps2 = pp_y.tile([128, BT], F32, name="ps2", tag="y")
                            for kc in range(2):
                                nc.tensor.matmul(
                                    ps2,
                                    w2s[:, 2 * kc : 2 * kc + 2, m * 128 : (m + 1) * 128],
                                    y1[t][:, 2 * kc : 2 * kc + 2, :],
                                    start=(kc == 0),
                                    stop=(kc == 1),
                                    perf_mode=DR,
                                )
                            drain1(y2[t][:, m, :], ps2, b2s, m, (m + i) % 2 == 1)
                    for i, t in enumerate(ts):
                        u = i
                        for kc in range(2):
                            nc.tensor.matmul(
                                z3q[0:64, :],
                                w3s[:, 2 * kc : 2 * kc + 2, 64 * u : 64 * u + 64],
                                y2[t][:, 2 * kc : 2 * kc + 2, :],
                                start=(u == 0 and kc == 0),
                                stop=False,
                                perf_mode=DR,
                            )
                    continue
                else:
                    for m in range(4):
                        ps1 = {
                            t: pp_y.tile([128, BT], F32, name="ps1", tag="y")
                            for t in ts
                        }
                        for t in ts:
                            nc.tensor.matmul(
                                ps1[t],
                                w1s[:, 0:2, m * 128 : (m + 1) * 128],
                                obsb[t],
                                start=True,
                                stop=True,
                                perf_mode=DR,
                            )
                        for i, t in enumerate(ts):
                            drain1(y1[t][:, m, :], ps1[t], b1s, m, (m + i) % 2 == 0)

                if pend is not None:
                    tail(*pend)
                    pend = None

                # layer 2: psum = 64*z2; drain y2' = relu(psum + 64*b2) = 64*y2
                y2 = {
                    t: p_y2.tile(
                        [128, 4, HID], FP8, name=f"y2_{t % 2}", tag=f"y2_{t % 2}"
                    )
                    for t in ts
                }
                for m in range(4):
                    ps2 = {
                        t: pp_y.tile([128, BT], F32, name="ps2", tag="y") for t in ts
                    }
                    for kc in range(2):
                        for t in ts:
                            nc.tensor.matmul(
                                ps2[t],
                                w2s[:, 2 * kc : 2 * kc + 2, m * 128 : (m + 1) * 128],
                                y1[t][:, 2 * kc : 2 * kc + 2, :],
                                start=(kc == 0),
                                stop=(kc == 1),
                                perf_mode=DR,
                            )
                    for i, t in enumerate(ts):
                        drain1(y2[t][:, m, :], ps2[t], b2s, m, (m + i) % 2 == 1)

                # layer 3: psum = 1024*z3 for both tiles in ONE bank (tile u
                # of the quad -> psum rows 16u:16u+16 -> concurrent
                # col-groups), then fused bias+sigmoid on ACT into a
                # quad-merged qpT [64, 512]
                # kc-outer: the two kc0 matmuls (which only need the m0/m1
                # y2 drains) run while the m2/m3 drains finish, so kc1 never
                # waits at the pair boundary
                for kc in range(2):
                    for i, t in enumerate(ts):
                        u = 2 * (g % 2) + i
                        nc.tensor.matmul(
                            z3q[0:64, :],
                            w3s[:, 2 * kc : 2 * kc + 2, 64 * u : 64 * u + 64],
                            y2[t][:, 2 * kc : 2 * kc + 2, :],
                            start=(u == 0 and kc == 0),
                            stop=(u == 3 and kc == 1),
                            perf_mode=DR,
                        )
                if g % 2 == 1:
                    qpT = p_qp.tile([64, BT], F32, tag="qpT")
                    nc.scalar.activation(
                        out=qpT,
                        in_=z3q,
                        func=AF.Sigmoid,
                        bias=b3s[:, 0:1],
                        scale=1.0 / 1024.0,
                    )
                    pend = (g // 2, qpT, u0b[g // 2])
            tail(*pend)
    nc.finalize()
    return nc


def _get_nc(zero_bias):
    key = ("nc", zero_bias)
    if key not in _CACHE:
        _CACHE[key] = _build_nc(zero_bias)
    return _CACHE[key]


FP8NP = ml_dtypes.float8_e4m3  # TRN float8e4: bias 7, max normal +-240


def _to_fp8(x):
    return np.ascontiguousarray(np.clip(x, -240.0, 240.0)).astype(FP8NP)


def kernel(obs, x_init, u_init, W1, b1, W2, b2, W3, b3):
    obs = np.asarray(obs, dtype=np.float32)
    u_init = np.ascontiguousarray(np.asarray(u_init, dtype=np.float32))
    W1 = np.asarray(W1, dtype=np.float32)
    W2 = np.asarray(W2, dtype=np.float32)
    W3 = np.asarray(W3, dtype=np.float32)
    b1 = np.asarray(b1, dtype=np.float32)
    b2 = np.asarray(b2, dtype=np.float32)
    b3 = np.asarray(b3, dtype=np.float32)

    # weights with fp8 scale folding (see module docstring)
    w1h = _to_fp8((8.0 * W1).reshape(2, 128, HID).transpose(1, 0, 2))
    w2h = _to_fp8((64.0 * W2).reshape(4, 128, HID).transpose(1, 0, 2))
    # four 64-wide W3 variants: tile u of a quad -> psum rows 16u:16u+16
    w3u = np.zeros((HID, 256), dtype=np.float32)
    for u in range(4):
        w3u[:, 64 * u + 16 * u : 64 * u + 16 * u + 4] = 16.0 * W3[:, 12:16]
        w3u[:, 64 * u + 16 * u + 4 : 64 * u + 16 * u + 8] = 16.0 * W3[:, 28:32]
    w3h = _to_fp8(w3u.reshape(4, 128, 256).transpose(1, 0, 2))
    b1p = np.ascontiguousarray(b1.reshape(4, 128).T)
    b2p = np.ascontiguousarray(64.0 * b2.reshape(4, 128).T)
    b3p = np.zeros((64, 1), dtype=np.float32)
    for u in range(4):
        b3p[16 * u : 16 * u + 4, 0] = b3[12:16]
        b3p[16 * u + 4 : 16 * u + 8, 0] = b3[28:32]
    idp = np.eye(64, dtype=np.float32)

    zero_bias = bool(np.all(b1 == 0.0) and np.all(b2 == 0.0))
    nc = _get_nc(zero_bias)
    in_maps = []
    for i in range(NCORES):
        oc = obs[i * BPC : (i + 1) * BPC]  # [4096, 256]
        # [t, p, kc, n] = obs[t*512+n, kc*128+p] / 8
        obsT = _to_fp8(oc.reshape(NT, BT, 2, 128).transpose(0, 3, 2, 1) / 8.0)
        # u0[G, p, c, u, j] = u_init[(4G+u)*512 + c*128 + p, j]
        u0c = u_init[i * BPC : (i + 1) * BPC].reshape(NT // 4, 4, 4, 128, 4)
        u0c = np.ascontiguousarray(u0c.transpose(0, 3, 2, 1, 4))
        in_maps.append(
            {
                "obsT": obsT,
                "u0": u0c,
                "w1": w1h,
                "w2": w2h,
                "w3": w3h,
                "b1": b1p,
                "b2": b2p,
                "b3": b3p,
                "idm": idp,
            }
        )
    import os

    kw = {}
    if os.environ.get("BASSK_TRACE"):
        kw = {"trace": True, "tmpdir": os.environ.get("BASSK_TRACE_DIR") or None}
    res = run_bass_kernel_spmd(nc, in_maps, list(range(NCORES)), **kw)
    _CACHE["last_result"] = res
    outs = []
    for i in range(NCORES):
        arr = np.asarray(res.results[i]["uo"]).astype(np.float32)  # [g,p,c,u,j]
        outs.append(arr.transpose(0, 3, 2, 1, 4).reshape(BPC, 4))
    return np.concatenate(outs, axis=0).astype(np.float32)
